# revision 1
# baseline (speedup 1.0000x reference)
"""Point-Transformer block as a Bass/Tile kernel for 8 Trainium2 NeuronCores.

Strategy
--------
Data-parallel over batch: core b handles batch element b (B == 8 == n_cores).

Host-side algebraic folding (all on 64x64-ish weights, negligible cost):
  * x1/x2 are never materialized: the gate-logit path
      h = relu(Ww1 @ reshape(x1 - x2 + ptf))
    folds into per-j 64->8 matmuls with folded weights D_j = -Ww1_j@W2
    (+ Ww1-rowsum@W1 for j==0), plus a ptsn term and a constant.
  * pt_conv collapses: ptsn >= 0 so relu(Wp1*ptsn) = relu(Wp1)*ptsn, hence
      ptf = v (x) ptsn  with  v = Wp2 @ relu(Wp1).
  * b3 is deferred through the softmax (sum_k ws = 1).
  * ptsn (the 3-coordinate squared distance, 0.03% of total FLOPs) is
    computed on the host directly in the transposed [j, n] layout the PE
    consumes for the rank-1 v (x) ptsn accumulations.

All matmuls run as float32r (TF32-like fast mode, 1 col/cycle vs 4 for
strict fp32).  On-chip layout: channels on partitions, the two 256-point
halves of each 512-point tile packed into partitions [0:64)/[64:128) so the
64-channel matmuls run at full 128-partition occupancy (block-diagonal
weights).  x3 / gate weights are k-major [128, 10, 256].  Softmax
normalization: unnormalized exp, then a reciprocal of the 16-row sum
broadcast back to 128 rows with a 0/1 selection matmul.
"""

import numpy as np

B, N, K = 8, 8192, 10
CH = 64          # IN == MID == OUT
SP = 8
GN = CH // SP    # 8 gate channels
TN = 512         # points per tile
TN2 = TN // 2    # points per partition-half
NT = N // TN     # 16 tiles

_CACHE = {}


def _build_bass():
    import concourse.bacc as bacc
    import concourse.tile as tile
    from concourse import mybir

    import os
    f32 = mybir.dt.float32
    f32r = mybir.dt.float32r
    if os.environ.get("NO_F32R"):
        f32r = f32
    AF = mybir.ActivationFunctionType
    OP = mybir.AluOpType

    nc = bacc.Bacc("TRN2", target_bir_lowering=False)

    def mm(out, lhsT, rhs, **kw):
        nc.tensor.matmul(out, lhsT, rhs, **kw)

    # ---------------- DRAM I/O ----------------
    # feats pre-packed on host: [c + 64*half, tile*2560 + within]
    feats_d = nc.dram_tensor("feats", [128, N * K // 2], f32r,
                             kind="ExternalInput")
    # host-computed ptsn, transposed per tile: [tile, j + 10*half, n2]
    ptT_d = nc.dram_tensor("ptT", [NT, 2 * K, TN2], f32r,
                             kind="ExternalInput")
    out_d = nc.dram_tensor("out", [CH, N], f32, kind="ExternalOutput")

    cshapes = {
        "w0ddT": [128, 128], "b0dd": [128, 1], "w3ddT": [128, 128],
        "dls": [128, 10 * 16], "w1vdd": [20, 16], "cbdd": [16, 1],
        "vk": [20, 10 * 128], "wkat": [33, 10 * 128],
        "ww2A": [16, 80], "ww2B": [16, 80], "bw2col": [80, 1],
        "s8selA": [80, 16], "s8selB": [80, 16], "obc": [16, 128],
        "woutddT": [128, 128], "idd": [128, 128],
        "b3dd": [128, 1], "boutdd": [128, 1], "hauginit": [33, TN2],
    }
    F32_CONSTS = {"b0dd", "cbdd", "b3dd", "boutdd", "bw2col", "obc"}
    consts_d = {k: nc.dram_tensor(k, v, f32 if k in F32_CONSTS else f32r,
                                  kind="ExternalInput")
                for k, v in cshapes.items()}

    with tile.TileContext(nc) as tc:
        with (
            tc.tile_pool(name="singles", bufs=1) as singles,
            tc.tile_pool(name="bigio", bufs=2) as bigio,
            tc.tile_pool(name="work", bufs=2) as work,
            tc.tile_pool(name="small", bufs=3) as small,
            tc.tile_pool(name="xnps", bufs=3, space="PSUM") as xnps_pool,
            tc.tile_pool(name="grpps", bufs=2, space="PSUM") as grpps_pool,
            tc.tile_pool(name="sbank", bufs=1, space="PSUM") as sbank_pool,
        ):
            # ---- persistent consts in SBUF ----
            csb = {}
            for name, shp in cshapes.items():
                dt_ = f32 if name in F32_CONSTS else f32r
                t = singles.tile(shp, dt_, name=f"c_{name}")
                nc.sync.dma_start(out=t, in_=consts_d[name][:, :])
                csb[name] = t

            # alternating persistent gate tiles: rows 0-15 h, row 32 ones
            h_augs = []
            for p in range(2):
                ht = singles.tile([33, TN2], f32r, name=f"haug{p}")
                nc.sync.dma_start(out=ht, in_=consts_d["hauginit"][:, :])
                h_augs.append(ht)

            # persistent small PSUM banks (all matmul outs at partition 0)
            SB1 = sbank_pool.tile([128, 512], f32, name="SB1")
            SB2 = sbank_pool.tile([128, 512], f32, name="SB2")
            SB3 = sbank_pool.tile([128, 512], f32, name="SB3")
            G_ps = SB1[0:16, 0:TN2]
            s8_ps = SB1[0:16, TN2:2 * TN2]
            wv_ps = SB2[0:80, 0:TN2]         # one half at a time
            rsb_ps = SB3[0:128, 0:TN2]
            out2_ps = SB3[:, TN2:2 * TN2]

            XCH = [(i * 500, 500) for i in range(5)] + [(2500, 60)]
            KGROUPS = [(0, 2), (2, 2), (4, 2), (6, 2), (8, 2)]

            pending_head = None
            for it in range(NT):
                n0 = it * TN
                h_aug = h_augs[it % 2]

                # ---------- input DMAs ----------
                feats_t = bigio.tile([128, TN2 * K], f32r, name="feats_t")
                nc.sync.dma_start(
                    out=feats_t,
                    in_=feats_d[:, it * TN2 * K:(it + 1) * TN2 * K])
                ptT20 = small.tile([2 * K, TN2], f32r, name="ptT20")
                nc.sync.dma_start(out=ptT20, in_=ptT_d[it])

                # ---------- xn = relu(W0 @ feats + b0), k-major in SBUF ----
                xn_sb = work.tile([128, K, TN2], f32r, name="xn_sb")
                xnv = xn_sb.rearrange("p k n -> p n k")
                for ci, (off, sz) in enumerate(XCH):
                    xn_ps = xnps_pool.tile([128, 512], f32, name="xn_ps",
                                           tag="xnps")
                    mm(xn_ps[:, :sz], csb["w0ddT"], feats_t[:, off:off + sz],
                       start=True, stop=True)
                    src = xn_ps[:, :sz].rearrange("p (n k) -> p n k", k=K)
                    dst = xnv[:, off // K:(off + sz) // K, :]
                    if ci in (0, 1, 2, 4):
                        nc.scalar.activation(
                            out=dst, in_=src, func=AF.Relu, bias=csb["b0dd"])
                    else:
                        nc.vector.tensor_scalar(
                            out=dst, in0=src, scalar1=csb["b0dd"], scalar2=0.0,
                            op0=OP.add, op1=OP.max)

                if pending_head is not None:
                    pending_head()
                    pending_head = None

                # ---------- gate logits G [16, 256] -> h ----------
                for j in range(K):
                    mm(G_ps, csb["dls"][:, 16 * j:16 * (j + 1)],
                       xn_sb[:, j, :], start=(j == 0), stop=False)
                mm(G_ps, csb["w1vdd"], ptT20, start=False, stop=True)
                nc.scalar.activation(
                    out=h_aug[0:16, :], in_=G_ps, func=AF.Relu,
                    bias=csb["cbdd"])

                # ---------- softmax denominator -> 1/s broadcast ----------
                e_sb = work.tile([80, 512], f32r, name="e_sb")
                mm(wv_ps, csb["ww2A"], h_aug[0:16, :], start=True, stop=True)
                nc.scalar.activation(
                    out=e_sb[:, 0:TN2], in_=wv_ps, func=AF.Exp,
                    bias=csb["bw2col"])
                mm(wv_ps, csb["ww2B"], h_aug[0:16, :], start=True, stop=True)
                nc.scalar.activation(
                    out=e_sb[:, TN2:2 * TN2], in_=wv_ps, func=AF.Exp,
                    bias=csb["bw2col"])
                mm(s8_ps, csb["s8selA"], e_sb[:, 0:TN2], start=True,
                   stop=False)
                mm(s8_ps, csb["s8selB"], e_sb[:, TN2:2 * TN2], start=False,
                   stop=True)
                s8_sb = small.tile([16, TN2], f32, name="s8_sb")
                nc.scalar.activation(out=s8_sb, in_=s8_ps, func=AF.Identity,
                                     bias=0.0)
                rs8_sb = small.tile([16, TN2], f32, name="rs8_sb")
                scr_sb = small.tile([16, TN2], f32, name="scr_sb")
                nc.vector.reciprocal_approx_accurate(
                    out=rs8_sb, in_=s8_sb, scratch=scr_sb)
                nc.tensor.matmul(rsb_ps, csb["obc"], rs8_sb, start=True,
                                 stop=True)

                # ---------- per-k-group: gate weights & x3, product ----
                ws_sb = work.tile([128, K, TN2], f32, name="ws_sb")
                y_sb = work.tile([128, K, TN2], f32, name="y_sb")
                for (k0, kg) in KGROUPS:
                    ws_ps = grpps_pool.tile([128, 2, TN2], f32, name="ws_ps",
                                            tag="grp")
                    for i in range(kg):
                        k = k0 + i
                        mm(ws_ps[:, i, :],
                           csb["wkat"][:, 128 * k:128 * (k + 1)],
                           h_aug, start=True, stop=True)
                    nc.scalar.activation(
                        out=ws_sb[:, k0:k0 + kg, :], in_=ws_ps[:, 0:kg, :],
                        func=AF.Exp)

                    x3_ps = grpps_pool.tile([128, 2, TN2], f32, name="x3_ps",
                                            tag="grp")
                    # NOTE: start=True clears has_written for the WHOLE bank,
                    # so each slice's accumulation group must complete before
                    # the next slice starts.
                    for i in range(kg):
                        k = k0 + i
                        mm(x3_ps[:, i, :], csb["w3ddT"], xn_sb[:, k, :],
                           start=True, stop=False)
                        mm(x3_ps[:, i, :],
                           csb["vk"][:, 128 * k:128 * (k + 1)],
                           ptT20, start=False, stop=True)
                    nc.vector.tensor_tensor(
                        out=y_sb[:, k0:k0 + kg, :],
                        in0=ws_sb[:, k0:k0 + kg, :], in1=x3_ps[:, 0:kg, :],
                        op=OP.mult)

                # ---------- weighted sum over k, normalize ----------
                num_sb = small.tile([128, TN2], f32, name="num_sb")
                nc.vector.tensor_reduce(
                    out=num_sb, in_=y_sb.rearrange("p k n -> p n k"),
                    axis=mybir.AxisListType.X, op=OP.add)
                o1p_sb = small.tile([128, TN2], f32, name="o1p_sb")
                nc.vector.scalar_tensor_tensor(
                    out=o1p_sb, in0=num_sb, scalar=0.0, in1=rsb_ps,
                    op0=OP.bypass, op1=OP.mult)
                o1_sb = small.tile([128, TN2], f32r, name="o1_sb")
                nc.scalar.activation(
                    out=o1_sb, in_=o1p_sb, func=AF.Relu, bias=csb["b3dd"])

                def head(o1_sb=o1_sb, xn_sb=xn_sb, n0=n0):
                    mm(out2_ps, csb["woutddT"], o1_sb, start=True, stop=False)
                    mm(out2_ps, csb["idd"], xn_sb[:, 0, :], start=False,
                       stop=True)
                    fin_sb = small.tile([128, TN2], f32, name="fin_sb")
                    nc.scalar.activation(
                        out=fin_sb, in_=out2_ps, func=AF.Identity,
                        bias=csb["boutdd"])
                    nc.sync.dma_start(out=out_d[:, n0:n0 + TN2],
                                      in_=fin_sb[0:64, :])
                    nc.sync.dma_start(out=out_d[:, n0 + TN2:n0 + TN],
                                      in_=fin_sb[64:128, :])
                pending_head = head

            pending_head()

    nc.compile()
    return nc


def _fold_weights(inp):
    """Host-side weight folding -> dict of const arrays (all float32)."""
    W0, b0 = inp["W0"], inp["b0"]
    W1, b1 = inp["W1"], inp["b1"]
    W2, b2 = inp["W2"], inp["b2"]
    W3, b3 = inp["W3"], inp["b3"]
    Wp1, Wp2 = inp["Wp1"], inp["Wp2"]
    Ww1, Ww2, bw2 = inp["Ww1"], inp["Ww2"], inp["bw2"]
    Wout, bout = inp["Wout"], inp["bout"]

    Ww1r = Ww1.reshape(GN, CH, K)
    A = Ww1r.sum(axis=2)
    AW1 = A @ W1
    C2 = np.einsum("omj,mc->ocj", Ww1r, W2)
    Dc = -C2.copy()
    Dc[:, :, 0] += AW1
    cb = A @ (b1 - b2)
    v = Wp2 @ np.maximum(Wp1[:, 0], 0.0)
    w1v = np.einsum("omj,m->oj", Ww1r, v)

    m64 = np.arange(CH)

    c = {}
    t = np.zeros((128, 128), np.float32)
    t[0:64, 0:64] = W0.T; t[64:128, 64:128] = W0.T
    c["w0ddT"] = t
    c["b0dd"] = np.concatenate([b0, b0]).reshape(128, 1)
    t = np.zeros((128, 128), np.float32)
    t[0:64, 0:64] = W3.T; t[64:128, 64:128] = W3.T
    c["w3ddT"] = t
    t = np.zeros((128, 10 * 16), np.float32)
    for j in range(K):
        t[0:64, 16 * j:16 * j + 8] = Dc[:, :, j].T
        t[64:128, 16 * j + 8:16 * j + 16] = Dc[:, :, j].T
    c["dls"] = t
    t = np.zeros((20, 16), np.float32)
    for j in range(K):
        t[j, 0:8] = w1v[:, j]
        t[10 + j, 8:16] = w1v[:, j]
    c["w1vdd"] = t
    c["cbdd"] = np.concatenate([cb, cb]).reshape(16, 1).astype(np.float32)
    t = np.zeros((20, 10 * 128), np.float32)
    for k in range(K):
        t[k, 128 * k:128 * k + 64] = v
        t[10 + k, 128 * k + 64:128 * k + 128] = v
    c["vk"] = t
    # wkat [33, 128] per k: rows 0-15 Ww2 (block-diag), row 32 bw2
    t = np.zeros((33, 10 * 128), np.float32)
    for k in range(K):
        blk = np.zeros((33, 128), np.float32)
        for h in range(2):
            blk[8 * h:8 * h + 8, 64 * h:64 * h + 64] = \
                Ww2[(m64 % SP) * K + k].T
            blk[32, 64 * h:64 * h + 64] = bw2[(m64 % SP) * K + k]
        t[:, 128 * k:128 * (k + 1)] = blk
    c["wkat"] = t
    t = np.zeros((16, 80), np.float32); t[0:8, :] = Ww2.T
    c["ww2A"] = t
    t = np.zeros((16, 80), np.float32); t[8:16, :] = Ww2.T
    c["ww2B"] = t
    c["bw2col"] = bw2.reshape(80, 1).astype(np.float32)
    t = np.zeros((80, 16), np.float32)
    for g in range(SP):
        for j in range(K):
            t[g * K + j, g] = 1.0
    c["s8selA"] = t
    t = np.zeros((80, 16), np.float32)
    for g in range(SP):
        for j in range(K):
            t[g * K + j, 8 + g] = 1.0
    c["s8selB"] = t
    t = np.zeros((16, 128), np.float32)
    for h in range(2):
        t[(m64 % SP) + 8 * h, m64 + 64 * h] = 1.0
    c["obc"] = t
    t = np.zeros((128, 128), np.float32)
    t[0:64, 0:64] = Wout.T; t[64:128, 64:128] = Wout.T
    c["woutddT"] = t
    c["idd"] = np.eye(128, dtype=np.float32)
    t = np.zeros((33, TN2), np.float32); t[32, :] = 1.0
    c["hauginit"] = t
    c["b3dd"] = np.concatenate([b3, b3]).reshape(128, 1)
    c["boutdd"] = np.concatenate([bout, bout]).reshape(128, 1)
    return c


def make_in_maps(inputs):
    inp = {k: np.ascontiguousarray(np.asarray(v, dtype=np.float32))
           for k, v in inputs.items()}
    consts = _fold_weights(inp)
    # host ptsn for all cores at once: [B, N, K]
    cent = inp["cent_pts"]                      # [B, N, 3]
    spt = inp["sm_pts"]                         # [B, 3, N, K]
    ptsn = ((cent.transpose(0, 2, 1)[:, :, :, None] - spt) ** 2).sum(axis=1)
    in_maps = []
    for b in range(B):
        m = dict(consts)
        ff = inp["sm_feats"][b].reshape(CH, NT, 2, TN2 * K)
        m["feats"] = np.ascontiguousarray(
            np.concatenate([ff[:, :, 0, :], ff[:, :, 1, :]], axis=0)
        ).reshape(128, N * K // 2)
        m["ptT"] = np.ascontiguousarray(
            ptsn[b].reshape(NT, 2, TN2, K).transpose(0, 1, 3, 2)
        ).reshape(NT, 2 * K, TN2)
        in_maps.append(m)
    return in_maps


def _run(inputs, trace=False):
    from concourse.bass_utils import run_bass_kernel_spmd

    if "nc" not in _CACHE:
        _CACHE["nc"] = _build_bass()
    nc = _CACHE["nc"]
    in_maps = make_in_maps(inputs)

    res = run_bass_kernel_spmd(
        nc, in_maps, core_ids=list(range(B)), trace=trace)
    out = np.stack([r["out"] for r in res.results], axis=0)
    return out, res


def kernel(**inputs) -> np.ndarray:
    out, _ = _run(inputs, trace=False)
    return out

